# revision 49
# baseline (speedup 1.0000x reference)
"""Trainium2 Bass kernel for nn_MEX_41386304864396 (dense transformer block).

Sharding: data-parallel over batch B=8 across 8 NeuronCores (one batch element
per core); weights replicated.  Host pre-transposes activations to [D, S] and
pre-folds constants so the device never transposes:
  * residual fold      W' = I + W                  (embed blocks LN(x + xW + b))
  * LN mean fold       extra matmul column: m = x @ (W'.sum(1)/D) + mean(b)
  * LN affine fold     gamma/beta folded into the downstream q/k/v weights
  * v-bias fold        bd' = bd + (bv_g + bv_p) @ Wd   (softmax rows sum to 1)
  * bridged attention  glb_ctx + plb_ctx = softmax_g @ (vg + softmax_p @ vp)
    (associativity removes the S x S x S 'enhanced' matmul)
Scores are computed transposed [key, query]; softmax uses exp without max
subtraction (scores bounded ~|3.4|) and normalization is deferred:
Z_p is a folded ones-column of the U_p matmul, Z_g a folded ones-row of U_g
(landed by a zero-column in vg so Z*(1/Z)+0 writes the ones-row for free);
1/Z_g and the LN 1/std rows are partition-broadcast on the idle GpSimd
engine.  exp(scores_p) for all 16 heads is precomputed in fp8e4m3 storage,
woven into the kg/vg projection loops so the exps drain on the Activation
engine underneath projection matmuls.
All matmul operands are bfloat16 (PSUM accumulation fp32; LN statistics and
biases fp32), halving HBM traffic and running the PE at full rate even for
narrow outputs.  Each weight matrix is loaded with a single DMA from a host
pre-arranged [128, K-chunks x cols] layout (contiguous per partition line),
through shared double-buffered pools so the next phase's weights stream in
during the current phase's matmuls.  SBUF pools are two-sided LIFO stacks
scoped to phases to stay inside the partition budget.
"""
import os
import sys

sys.path.insert(0, '/opt/trn_rl_repo')

import numpy as np
import ml_dtypes

import concourse.bass as bass  # noqa: F401
import concourse.tile as tile
from concourse import bacc, mybir
from concourse import bass2jax

F32 = mybir.dt.float32
BF16 = mybir.dt.bfloat16
F8 = mybir.dt.float8e4
AF = mybir.ActivationFunctionType
ALU = mybir.AluOpType

S, B, D, H, DH, FF = 512, 8, 1024, 16, 64, 4096
NK = D // 128
NT = S // 128
NFF = FF // 128
EPS = 1e-5
SCALE = 1.0 / 8.0
NPRE = 16  # heads with exp(scores_p) precomputed under the projections

DT = BF16
NPDT = ml_dtypes.bfloat16

# bcols column map: [l1 x8 | l2 x8 | g x8 | qg x8 | kg x8 | qp x8 | kp x8 |
#                    d x8 | proj x8 | fc x32]
BCOL_GROUPS = ["l1", "l2", "g", "qg", "kg", "qp", "kp", "d", "proj"]
BCOL_OFF = {n: 8 * i for i, n in enumerate(BCOL_GROUPS)}
BCOL_FC_OFF = 8 * len(BCOL_GROUPS)
BCOL_W = BCOL_FC_OFF + NFF


def _declare(nc, timing=False):
    dram = {}
    kind = "Internal" if timing else "ExternalInput"

    def din(name, shape, dt=DT):
        dram[name] = nc.dram_tensor(name, list(shape), dt, kind=kind)

    for n in ("xg", "xl", "xt"):
        din(n, (128, NK, S))
    for n in ("we_l1", "we_l2", "we_g", "w_qg", "w_kg", "w_qp", "w_kp",
              "w_vg", "w_vp", "w_d", "w_ml"):
        din(n, (128, NK, D))
    din("w_fc", (4, 128, NFF // 4, NK, 128))
    din("w_proj", (4, 128, NK // 4, NFF, 128))
    din("wmeans", (128, 3 * NK))
    din("bcols", (128, BCOL_W), F32)
    din("bml_bc", (128, D), F32)
    dram["y"] = nc.dram_tensor("y", [S, D], F32, kind="ExternalOutput")
    return dram


def _body(nc, tc, dram, mean_b):
    def pool(name, bufs, side="left", space="SBUF"):
        return tc.alloc_tile_pool(name=name, bufs=bufs, side=side, space=space)

    # ---- global pools ----
    consts = pool("consts", 1)
    rows = pool("rows", 1)
    tmp = pool("tmp", 2)
    small = pool("small", 3)

    psA = pool("psA", 4, space="PSUM")
    # psRow lives only through the embeds; psUp/psUg only through attention.
    # Scoping them keeps the total inside 8 PSUM banks with 2 bufs each.
    psRow = pool("psRow", 4, space="PSUM")

    def mmtile():
        return psA.tile([128, 512], F32, tag="mm", name="mm")

    # ---- constants ----
    ones_f = consts.tile([128, 32], F32, tag="ones_f", name="ones_f")
    nc.vector.memset(ones_f[:], 1.0)
    ones_dt = consts.tile([128, 32], DT, tag="ones_dt", name="ones_dt")
    nc.vector.tensor_copy(ones_dt[:], ones_f[:])
    onesr_f = consts.tile([1, 128], F32, tag="onesr_f", name="onesr_f")
    nc.vector.memset(onesr_f[:], 1.0)
    ones_row = consts.tile([1, 128], DT, tag="ones_row", name="ones_row")
    nc.vector.tensor_copy(ones_row[:], onesr_f[:])
    eps_t = consts.tile([1, 1], F32, tag="eps_t", name="eps_t")
    nc.vector.memset(eps_t[:], EPS)

    wmeans = consts.tile([128, 3 * NK], DT, tag="wmeans", name="wmeans")
    nc.sync.dma_start(out=wmeans[:], in_=dram["wmeans"].ap())
    bcols = consts.tile([128, BCOL_W], F32, tag="bcols", name="bcols")
    nc.sync.dma_start(out=bcols[:], in_=dram["bcols"].ap())

    def bcol(group, j):
        return bcols[:, BCOL_OFF[group] + j:BCOL_OFF[group] + j + 1]

    def bcol_fc(j):
        return bcols[:, BCOL_FC_OFF + j:BCOL_FC_OFF + j + 1]

    def xload(dname, p, tagp):
        t = p.tile([128, NK, S], DT, tag=tagp, name=tagp)
        nc.sync.dma_start(out=t[:], in_=dram[dname].ap())
        return t

    def wload(wp, name):
        t = wp.tile([128, NK, D], DT, tag="w", name=name)
        nc.sync.dma_start(out=t[:], in_=dram[name].ap())
        return t

    # ---- embeds (plain LN; gamma/beta folded downstream on host) ----
    e_state = {}

    def embedA(e, eidx, x, wt, pyln):
        mp = psRow.tile([1, 512], F32, tag="row", name="mp")
        for k in range(NK):
            nc.tensor.matmul(mp[:], wmeans[:, eidx * NK + k:eidx * NK + k + 1],
                             x[:, k, :], start=(k == 0), stop=(k == NK - 1))
        ss = psRow.tile([1, 512], F32, tag="row", name="ss")
        ys = []
        pend = []
        for m in range(NK):
            ps = mmtile()
            for j in range(NK):
                k = (m + 1 + j) % NK
                nc.tensor.matmul(ps[:], wt[:, k, m * 128:(m + 1) * 128],
                                 x[:, k, :], start=(j == 0), stop=(j == NK - 1))
            y = pyln.tile([128, 512], DT, tag=f"y{m}", name=f"y{m}")
            nc.vector.tensor_scalar_add(y[:], ps[:], bcol(e, m))
            s = sq.tile([128, 512], DT, tag="sq", name="sq")
            nc.scalar.activation(s[:], ps[:], AF.Square,
                                 bias=bcol(e, m), scale=1.0)
            pend.append(s)
            if m > 0:
                s0 = pend.pop(0)
                nc.tensor.matmul(ss[:], ones_dt[:, 0:1], s0[:],
                                 start=(m == 1), stop=False)
            ys.append(y)
        s0 = pend.pop(0)
        nc.tensor.matmul(ss[:], ones_dt[:, 0:1], s0[:], start=False, stop=True)
        e_state[e] = (mp, ss, ys)

    def embedB_stats(e):
        mp, ss, ys = e_state[e]
        m_sb = erows.tile([1, 512], F32, tag=f"m_sb{e}", name="m_sb")
        nc.vector.tensor_scalar_add(m_sb[:], mp[:], float(mean_b[e]))
        msq = erows.tile([1, 512], F32, tag=f"msq{e}", name="msq")
        nc.vector.tensor_mul(msq[:], m_sb[:], m_sb[:])
        var = erows.tile([1, 512], F32, tag=f"var{e}", name="var")
        nc.vector.scalar_tensor_tensor(out=var[:], in0=ss[:], scalar=1.0 / D,
                                       in1=msq[:], op0=ALU.mult, op1=ALU.subtract)
        std = erows.tile([1, 512], F32, tag=f"std{e}", name="std")
        nc.scalar.activation(std[:], var[:], AF.Sqrt, bias=eps_t[:], scale=1.0)
        rstd = erows.tile([1, 512], F32, tag=f"rstd{e}", name="rstd")
        nc.vector.reciprocal(rstd[:], std[:])
        mr = erows.tile([1, 512], F32, tag=f"mr{e}", name="mr")
        nc.vector.tensor_mul(mr[:], m_sb[:], rstd[:])
        rstd_b = erows.tile([1, 512], DT, tag=f"rstd_b{e}", name="rstd_b")
        nc.vector.tensor_copy(rstd_b[:], rstd[:])
        mr_b = erows.tile([1, 512], DT, tag=f"mr_b{e}", name="mr_b")
        nc.vector.tensor_copy(mr_b[:], mr[:])
        e_state[e] = (ys, rstd_b, mr_b)

    def embedB(e, epool):
        ys, rstd_b, mr_b = e_state[e]
        # broadcast 1/std and m/std down all partitions once per embed, via
        # replicating DMA on the (idle) Pool queue: no PE/PSUM/Act involved
        a_bc = pbc.tile([128, 512], DT, tag="a_bc", name="a_bc")
        nc.gpsimd.partition_broadcast(a_bc[:], rstd_b[:])
        b_bc = pbc.tile([128, 512], DT, tag="b_bc", name="b_bc")
        nc.gpsimd.partition_broadcast(b_bc[:], mr_b[:])
        et = []
        for m in range(NK):
            t1 = tmp.tile([128, 512], F32, tag="t1", name="t1")
            nc.vector.tensor_mul(t1[:], ys[m][:], a_bc[:])
            em = epool.tile([128, 512], DT, tag=f"e{m}", name=f"e{m}")
            nc.vector.tensor_sub(em[:], t1[:], b_bc[:])
            et.append(em)
        e_state[e] = et

    # RIGHT side, alloc order = reverse lifetime: pe_dup dies first (after
    # qp), then the embed-scratch pools on top die right after the embeds
    pe_g2 = pool("pe_g2", 1, side="right")
    pe_l2 = pool("pe_l2", 1, side="right")
    px_t = pool("px_t", 1, side="right")
    pe_dup = pool("pe_dup", 1, side="right")
    erows = pool("erows", 1, side="right")
    pbc = pool("pbc", 1, side="right")
    sq = pool("sqp", 3, side="right")
    w1a = pool("w1a", 2, side="right")
    px_gl = pool("px_gl", 1, side="right")
    pyln = pool("pyln", 2, side="right")

    # po1/pwk live below the embed scratch: reserving them now lets the
    # kp weight stream in during the embeds (no WAR on freed scratch)
    po1 = pool("po1", 1)
    pwk = pool("pwk", 1)
    pkp = pool("pkp", 1)

    # DMA issue order = first-use order: xl/w_l1 gate the first matmuls,
    # xt isn't needed until the kp projection
    xl = xload("xl", px_gl, "xl")
    w_l1 = wload(w1a, "we_l1")
    xg = xload("xg", px_gl, "xg")
    w_l2 = wload(w1a, "we_l2")
    xt = xload("xt", px_t, "xt")

    # software-pipelined embeds: embedA(next) issues before embedB(cur) so
    # the PE never waits on a LN statistics chain
    wkpT = wload(pwk, "w_kp")
    embedA("l1", 0, xl, w_l1, pyln)
    embedA("l2", 1, xl, w_l2, pyln)
    w_g = wload(w1a, "we_g")
    embedB_stats("l1")
    embedB("l1", pe_dup)
    embedA("g", 2, xg, w_g, pyln)
    # stats chains issue before the bulk normalizes so the g chain is not
    # queued behind 16 DVE ops when its broadcast matmuls reach the PE
    embedB_stats("l2")
    embedB_stats("g")
    # kp projection (xt-fed, embed-independent) fills the PE while the
    # bulk LN normalizes drain on DVE/Pool
    xts = [xt[:, k, :] for k in range(NK)]
    kpT = projB("w_kp", xts, "kp", pkp, "kp", wt=wkpT)
    embedB("l2", pe_l2)
    embedB("g", pe_g2)
    dupT, l2T, g2T = e_state["l1"], e_state["l2"], e_state["g"]
    pyln.release()
    px_gl.release()
    w1a.release()
    sq.release()
    pbc.release()
    erows.release()
    psRow.release()

    def projB(wname, src, bgroup, opool, tagp, wt=None, mhook=None):
        if wt is None:
            wt = wload(pw, wname)
        out = []
        for m in range(NK):
            ps = mmtile()
            for j in range(NK):
                k = (m + 1 + j) % NK
                nc.tensor.matmul(ps[:], wt[:, k, m * 128:(m + 1) * 128],
                                 src[k], start=(j == 0), stop=(j == NK - 1))
            o = opool.tile([128, 512], DT, tag=f"{tagp}{m}", name=f"{tagp}{m}")
            nc.vector.tensor_scalar_add(o[:], ps[:], bcol(bgroup, m))
            out.append(o)
            if mhook is not None:
                mhook(m)
        return out

    def projA(wname, src, opool, tagp, fill, shook=None):
        # width DH+1: col DH is 1.0 for the U_p Z-column (vp) and 0.0 for vg,
        # so Z*(1/Z)+0 = 1 lands the Z_g ones-row for free in the same op
        wt = wload(pw, wname)
        out = []
        for rt in range(NT):
            vt = opool.tile([128, H, DH + 1], DT, tag=f"{tagp}{rt}", name=f"{tagp}{rt}")
            col = vt[:, :, DH:DH + 1].rearrange("p h one -> p (h one)")
            if fill == "ones":
                nc.vector.tensor_copy(col, ones_dt[:, 0:H])
            else:
                nc.vector.memset(col, 0.0)
            for half in range(2):
                ps = mmtile()
                for j in range(NK):
                    k = (2 * rt + half + 1 + j) % NK
                    nc.tensor.matmul(
                        ps[:], src[k][:, rt * 128:(rt + 1) * 128],
                        wt[:, k, half * 512:(half + 1) * 512],
                        start=(j == 0), stop=(j == NK - 1))
                nc.vector.tensor_copy(
                    vt[:, half * 8:(half + 1) * 8, 0:DH],
                    ps[:].rearrange("p (h d) -> p h d", h=8))
                if shook is not None:
                    shook(rt * 2 + half)
            out.append(vt)
        return out

    def eslices(et):
        return [et[k][:] for k in range(NK)]

    def hsl(tiles, h):
        return tiles[h // 2][64 * (h % 2):64 * (h % 2) + 64, :]

    # projection order: xt-fed first (no embed dependency), qg last so the
    # g-embed LN chain hides under kp/qp/vp/kg/vg matmuls.  Pool alloc order
    # is reverse lifetime: pkp/pqp die right after the p-score precompute.
    pqg = pool("pqg", 1)
    pkg = pool("pkg", 1)
    pv = pool("pv", 1)
    pqp = pool("pqp", 1)
    pw = pool("pw", 2)
    qpT = projB("w_qp", eslices(dupT), "qp", pqp, "qp")
    pe_dup.release()
    vp = projA("w_vp", xts, pv, "vpn", "ones")
    px_t.release()

    # precompute exp(scores_p) for all heads, one head woven into each
    # m-tile of the kg/qg projections: the 64 exps drain on the Activation
    # engine underneath projection matmuls, so neither the PE (psA bank
    # reuse) nor the attention loop waits on them
    pctx = pool("pctx", 1, side="right")
    pep = pool("pep", 1, side="right")
    ep_all = [None] * H

    def prescore(h):
        qp_h, kp_h = hsl(qpT, h), hsl(kpT, h)
        eps = []
        for kt in range(NT):
            sp = mmtile()
            nc.tensor.matmul(sp[:], kp_h[:, kt * 128:(kt + 1) * 128], qp_h,
                             start=True, stop=True)
            e = pep.tile([128, 512], F8, tag=f"pep{h}_{kt}", name=f"pep{h}_{kt}")
            nc.scalar.activation(e[:], sp[:], AF.Exp, scale=SCALE)
            eps.append(e)
        ep_all[h] = eps

    kgT = projB("w_kg", eslices(l2T), "kg", pkg, "kg", mhook=prescore)
    vg = projA("w_vg", eslices(l2T), pv, "vg", "zero",
               shook=lambda s: prescore(8 + s))
    qgT = projB("w_qg", eslices(g2T), "qg", pqg, "qg")
    pw.release()
    pwd = pool("pwd", 1)
    wdT = wload(pwd, "w_d")

    # ---- attention ----
    psUp = pool("psUp", 2, space="PSUM")
    psUg = pool("psUg", 2, space="PSUM")
    pexp = pool("pexp", 2, side="right")

    ctxT = [pctx.tile([128, 512], DT, tag=f"ctx{j}", name=f"ctx{j}")
            for j in range(NK)]
    stage1_out = {}

    def attn_stage1(h):
        qg_h, kg_h = hsl(qgT, h), hsl(kgT, h)
        if h < NPRE:
            ep = ep_all[h]
        else:
            qp_h, kp_h = hsl(qpT, h), hsl(kpT, h)
            ep = []
            for kt in range(NT):
                sp = mmtile()
                nc.tensor.matmul(sp[:], kp_h[:, kt * 128:(kt + 1) * 128], qp_h,
                                 start=True, stop=True)
                e = pexp.tile([128, 512], DT, tag=f"ep{kt}", name=f"ep{kt}")
                nc.scalar.activation(e[:], sp[:], AF.Exp, scale=SCALE)
                ep.append(e)
        eg = []
        for kt in range(NT):
            sg = mmtile()
            nc.tensor.matmul(sg[:], kg_h[:, kt * 128:(kt + 1) * 128], qg_h,
                             start=True, stop=True)
            e = pexp.tile([128, 512], DT, tag=f"eg{kt}", name=f"eg{kt}")
            nc.scalar.activation(e[:], sg[:], AF.Exp, scale=SCALE)
            eg.append(e)
        up = psUp.tile([128, NT, DH + 1], F32, tag="up", name="up")
        for kt in range(NT):
            for tt in range(NT):
                nc.tensor.matmul(up[:, kt, :], ep[tt][:, kt * 128:(kt + 1) * 128],
                                 vp[tt][:, h, :], start=(tt == 0),
                                 stop=(tt == NT - 1))
        rp = small.tile([128, NT], F32, tag="rp", name="rp")
        nc.vector.reciprocal(rp[:], up[:, :, DH])
        vph = []
        for kt in range(NT):
            vt = pexp.tile([128, DH + 1], DT, tag=f"vph{kt}", name=f"vph{kt}")
            nc.vector.scalar_tensor_tensor(
                out=vt[:], in0=up[:, kt, :], scalar=rp[:, kt:kt + 1],
                in1=vg[kt][:, h, :], op0=ALU.mult, op1=ALU.add)
            vph.append(vt)
        stage1_out[h] = (eg, vph)

    ug_state = {}

    def attn_s2a(h):
        eg, vph = stage1_out.pop(h)
        ug = psUg.tile([DH + 1, 512], F32, tag="ug", name="ug")
        for kt in range(NT):
            nc.tensor.matmul(ug[:], vph[kt][:], eg[kt][:],
                             start=(kt == 0), stop=(kt == NT - 1))
        ug_state[h] = ug

    def attn_s2b(h):
        ug = ug_state.pop(h)
        rg = rows.tile([1, 512], DT, tag="rg", name="rg")
        nc.vector.reciprocal(rg[:], ug[DH:DH + 1, :])
        rbs = tmp.tile([64, 512], DT, tag="t1", name="rbs")
        nc.gpsimd.partition_broadcast(rbs[:], rg[:])
        off = 64 * (h % 2)
        nc.vector.tensor_mul(ctxT[h // 2][off:off + 64, :], ug[0:DH, :], rbs[:])

    # s2b(h) is issued after stage1(h+2) so the rb matmul never stalls the
    # PE on the (DVE) 1/Z reciprocal; psUg bufs=2 decouples ug(h+1) from
    # the ctx(h) read
    attn_stage1(0)
    attn_stage1(1)
    attn_s2a(0)
    for h in range(H):
        if h + 2 < H:
            attn_stage1(h + 2)
        attn_s2b(h)
        if h + 1 < H:
            attn_s2a(h + 1)
    pexp.release()
    pep.release()

    # ---- out1 = ctx @ Wd + bd' ----
    out1T = projB("w_d", [c[:] for c in ctxT], "d", po1, "o1", wt=wdT)
    pwd.release()
    pqp.release()
    pv.release()
    pkg.release()
    pqg.release()
    pkp.release()
    pwk.release()
    pctx.release()
    pe_l2.release()
    pe_g2.release()

    # ---- MLP (alloc order = reverse release order: pwfc dies first) ----
    bml_bc = consts.tile([128, D], F32, tag="bml_bc", name="bml_bc")
    nc.sync.dma_start(out=bml_bc[:], in_=dram["bml_bc"].ap())
    ph1 = pool("ph1", 1)
    po2 = pool("po2", 1)
    pw3 = pool("pw3", 1)
    pwpj = pool("pwpj", 2)
    pwfc = pool("pwfc", 2)
    h1 = []
    for g in range(4):
        wt = pwfc.tile([128, NFF // 4, NK, 128], DT, tag="wfc", name="wfc")
        nc.sync.dma_start(out=wt[:], in_=dram["w_fc"].ap()[g])
        for j in range(NFF // 4):
            ff = g * (NFF // 4) + j
            ps = mmtile()
            for k in range(NK):
                nc.tensor.matmul(ps[:], wt[:, j, k, :], out1T[k],
                                 start=(k == 0), stop=(k == NK - 1))
            gl = ph1.tile([128, 512], DT, tag=f"h1_{ff}", name=f"h1_{ff}")
            nc.scalar.activation(gl[:], ps[:], AF.Gelu,
                                 bias=bcol_fc(ff), scale=1.0)
            h1.append(gl)
    pwfc.release()

    out2T = []
    for g in range(4):
        wt = pwpj.tile([128, NK // 4, NFF, 128], DT, tag="wpj", name="wpj")
        nc.sync.dma_start(out=wt[:], in_=dram["w_proj"].ap()[g])
        for i in range(NK // 4):
            m = g * (NK // 4) + i
            ps = mmtile()
            for k in range(NFF):
                nc.tensor.matmul(ps[:], wt[:, i, k, :], h1[k],
                                 start=(k == 0), stop=(k == NFF - 1))
            o = po2.tile([128, 512], DT, tag=f"o2m{m}", name=f"o2m{m}")
            nc.vector.tensor_scalar_add(o[:], ps[:], bcol("proj", m))
            out2T.append(o)
    pwpj.release()

    # ---- y = out2 @ Wml + bml (natural layout) ----
    outp = pool("outp", 2)
    wml = wload(pw3, "w_ml")
    for rt in range(NT):
        yt = outp.tile([128, D], F32, tag="yout", name="yout")
        for half in range(2):
            ps = mmtile()
            for j in range(NK):
                k = (2 * rt + half + 1 + j) % NK
                nc.tensor.matmul(ps[:], out2T[k][:, rt * 128:(rt + 1) * 128],
                                 wml[:, k, half * 512:(half + 1) * 512],
                                 start=(j == 0), stop=(j == NK - 1))
            nc.vector.tensor_add(yt[:, half * 512:(half + 1) * 512], ps[:],
                                 bml_bc[:, half * 512:(half + 1) * 512])
        nc.sync.dma_start(out=dram["y"].ap()[rt * 128:(rt + 1) * 128, :], in_=yt[:])
    outp.release()
    pw3.release()
    po2.release()
    ph1.release()
    po1.release()

    # (releases above follow LIFO: ..., po1, ph1, po2, pw3 stack order)

    for p in (small, tmp, rows, consts, psUg, psUp, psA):
        p.release()


def build(repeat=1, mean_b=None, timing=False):
    mean_b = mean_b or {"l1": 0.0, "l2": 0.0, "g": 0.0}
    nc = bacc.Bacc(None, target_bir_lowering=False, debug=False)
    dram = _declare(nc, timing=timing)
    with tile.TileContext(nc) as tc:
        with nc.allow_low_precision(reason="bf16 matmul operands, fp32 psum"):
            if repeat > 1:
                with tc.For_i(0, repeat, 1):
                    _body(nc, tc, dram, mean_b)
            else:
                _body(nc, tc, dram, mean_b)
    nc.compile()

    class CX:
        pass

    cx = CX()
    cx.nc = nc
    cx.dram = dram
    return cx


# ---------------------------------------------------------------------------
# host side
# ---------------------------------------------------------------------------

def _wlayout(W):
    """[D, C] -> [128, NK, C] with [p, k, n] = W[k*128+p, n], bf16."""
    C = W.shape[1]
    return np.ascontiguousarray(
        W.reshape(NK, 128, C).transpose(1, 0, 2)).astype(NPDT)


def _prep_host(inputs):
    f32 = np.float32
    g = np.asarray(inputs["global_feat"], f32)
    l = np.asarray(inputs["local_feat"], f32)
    t = np.asarray(inputs["text_feat"], f32)
    W = {k: np.asarray(inputs[k], f32) for k in
         ("Wg_emb", "Wl1", "Wl2", "Wq_g", "Wk_g", "Wv_g", "Wq_p", "Wk_p",
          "Wv_p", "Wd", "Wml", "Wfc", "Wproj")}
    bv = {k: np.asarray(inputs[k], f32) for k in
          ("bg_emb", "bl1", "bl2", "bq_g", "bk_g", "bv_g", "bq_p", "bk_p",
           "bv_p", "bd", "bml", "bproj", "bfc",
           "betag_emb", "betal1", "betal2", "gg_emb", "gl1", "gl2")}

    I = np.eye(D, dtype=f32)
    shared = {}
    mean_b = {}
    wmeans = np.zeros((128, 3 * NK), f32)
    bcols = np.zeros((128, BCOL_W), f32)
    for ei, (e, (wn, bn)) in enumerate(
            {"l1": ("Wl1", "bl1"), "l2": ("Wl2", "bl2"),
             "g": ("Wg_emb", "bg_emb")}.items()):
        Wp = (I + W[wn]).astype(f32)
        shared[f"we_{e}"] = _wlayout(Wp)
        wmeans[:, ei * NK:(ei + 1) * NK] = \
            (Wp.sum(axis=1) / D).astype(f32).reshape(NK, 128).T
        mean_b[e] = float(bv[bn].mean())
        bcols[:, BCOL_OFF[e]:BCOL_OFF[e] + NK] = bv[bn].reshape(NK, 128).T

    # fold LN gamma/beta of the producing embed into each consumer projection
    def foldp(Wname, bname, gamma, beta):
        Wf = (gamma[:, None] * W[Wname]).astype(f32)
        bf = (np.asarray(bv[bname]) + beta @ W[Wname]).astype(f32)
        return Wf, bf

    w_qg, b_qg = foldp("Wq_g", "bq_g", bv["gg_emb"], bv["betag_emb"])
    w_kg, b_kg = foldp("Wk_g", "bk_g", bv["gl2"], bv["betal2"])
    w_vg, b_vg = foldp("Wv_g", "bv_g", bv["gl2"], bv["betal2"])
    w_qp, b_qp = foldp("Wq_p", "bq_p", bv["gl1"], bv["betal1"])
    shared["w_qg"] = _wlayout(w_qg)
    shared["w_kg"] = _wlayout(w_kg)
    shared["w_qp"] = _wlayout(w_qp)
    shared["w_kp"] = _wlayout(W["Wk_p"])
    shared["w_vg"] = _wlayout(w_vg)
    shared["w_vp"] = _wlayout(W["Wv_p"])
    shared["w_d"] = _wlayout(W["Wd"])
    shared["w_ml"] = _wlayout(W["Wml"])
    for n, b_ in (("qg", b_qg), ("kg", b_kg), ("qp", b_qp),
                  ("kp", bv["bk_p"]), ("proj", bv["bproj"])):
        bcols[:, BCOL_OFF[n]:BCOL_OFF[n] + NK] = \
            np.asarray(b_, f32).reshape(NK, 128).T
    bdp = (bv["bd"] + (b_vg + bv["bv_p"]) @ W["Wd"]).astype(f32)
    bcols[:, BCOL_OFF["d"]:BCOL_OFF["d"] + NK] = bdp.reshape(NK, 128).T
    bcols[:, BCOL_FC_OFF:BCOL_FC_OFF + NFF] = bv["bfc"].reshape(NFF, 128).T
    shared["wmeans"] = np.ascontiguousarray(wmeans).astype(NPDT)
    shared["bcols"] = np.ascontiguousarray(bcols)
    shared["bml_bc"] = np.ascontiguousarray(
        np.broadcast_to(bv["bml"].reshape(1, D), (128, D)).astype(f32))
    # w_fc: [g, p, j, k, n] = Wfc[k*128+p, (g*8+j)*128+n]
    shared["w_fc"] = np.ascontiguousarray(
        W["Wfc"].reshape(NK, 128, 4, NFF // 4, 128).transpose(2, 1, 3, 0, 4)
    ).astype(NPDT)
    # w_proj: [g, p, i, k, n] = Wproj[k*128+p, (g*2+i)*128+n]
    shared["w_proj"] = np.ascontiguousarray(
        W["Wproj"].reshape(NFF, 128, 4, NK // 4, 128).transpose(2, 1, 3, 0, 4)
    ).astype(NPDT)

    def _xlayout(xb):
        # xb [S, D] -> [128, NK, S] with [p, k, s] = xb[s, k*128+p]
        return np.ascontiguousarray(
            xb.T.reshape(NK, 128, S).transpose(1, 0, 2)).astype(NPDT)

    in_maps = []
    for b in range(B):
        m = dict(shared)
        m["xg"] = _xlayout(g[:, b, :])
        m["xl"] = _xlayout(l[:, b, :])
        m["xt"] = _xlayout(t[:, b, :])
        in_maps.append(m)
    return in_maps, mean_b


_CACHE = {}


def get_built(repeat, mean_b):
    key = (repeat, tuple(sorted(mean_b.items())))
    if key not in _CACHE:
        _CACHE[key] = build(repeat=repeat, mean_b=mean_b)
    return _CACHE[key]


def run(inputs, repeat=1):
    in_maps, mean_b = _prep_host(inputs)
    cx = get_built(repeat, mean_b)
    results = bass2jax.run_bass_via_pjrt(cx.nc, in_maps, n_cores=B)
    return np.stack([results[b]["y"] for b in range(B)], axis=0)


def kernel(**inputs):
    return run(inputs, repeat=int(os.environ.get("BASS_NN_REPEAT", "1")))
